# revision 15
# baseline (speedup 1.0000x reference)
"""Trainium2 Bass kernel for nn_Decoder (embed -> LSTM -> vocab projection).

v3 layout (8 NeuronCores, single SPMD NEFF):
  - Host: embedding gather + concat -> lstm_in [B,T,DIN]; x_T [DIN, T*B]
    (col = t*B + b) replicated to all cores; weights pre-transposed.
  - LSTM is GATE-sharded (tensor parallel): core c owns hidden slice
    c*128:(c+1)*128, i.e. 4 gate-column chunks of 128 (i,f,g,o). Per step
    the core computes gates_T[512, 256] = W_hh_own @ h (full batch, M=128
    stationary chunks, N=256 moving) -> full PE width, 33 matmuls/step.
    gx contribution folded via identity-matmul; bias via ScalarE
    activation bias (per-partition). h_T_own [128,256] needs no
    transpose (already hidden-major).
  - Per-step AllGather of h_T_own (64KB) -> h_T full [1024,256]; the FC
    for step t-1 (hs[256,1024] @ fc_W_shard^T -> [256,3750]) runs on the
    PE while the AllGather for step t is in flight.
  - gx GEMM [512, 2560] = W_ih_own @ x^T done up front (160 matmuls,
    n-major so t=0's columns finish first).
  - Output logits [T*B, VL] fp32; host re-orders (t,b) -> (b,t) and
    concats vocab shards. No row permutation games.
"""
import ml_dtypes
import numpy as np
import jax
from jax.sharding import Mesh, PartitionSpec
from jax.experimental.shard_map import shard_map

import concourse.bass as bass
import concourse.mybir as mybir
import concourse.tile as tile
from concourse import bacc
from concourse.bass2jax import _bass_exec_p, install_neuronx_cc_hook, partition_id_tensor
from concourse.masks import make_identity

P = 128
NCORES = 8
B, T, FEAT, EMB, HID, VOCAB = 256, 10, 512, 512, 1024, 30000
DIN = FEAT + EMB          # 1024
KT = DIN // P             # 8 contraction tiles (DIN and HID both 1024)
GL = 4 * P                # 512 gate columns owned per core
VL = VOCAB // NCORES      # 3750 vocab per core
RA = T * B                # 2560 output rows (t-major)
NX = RA // 512            # 5 column chunks of x / gx
F32 = mybir.dt.float32
BF16 = mybir.dt.bfloat16
Act = mybir.ActivationFunctionType

# FC vocab chunks: 7 x 512 + 166
FC_CH = [(i * 512, min(512, VL - i * 512)) for i in range((VL + 511) // 512)]

_CACHE = {}


def _build_nc():
    nc = bacc.Bacc("TRN2", target_bir_lowering=False, debug=False, num_devices=NCORES)
    x_T = nc.dram_tensor("x_T", [DIN, RA], BF16, kind="ExternalInput").ap()
    wih_T = nc.dram_tensor("wih_T", [DIN, GL], BF16, kind="ExternalInput").ap()
    whh_T = nc.dram_tensor("whh_T", [HID, GL], BF16, kind="ExternalInput").ap()
    bias_t = nc.dram_tensor("bias_t", [P, 4], F32, kind="ExternalInput").ap()
    fc_wT = nc.dram_tensor("fc_wT", [HID, VL], BF16, kind="ExternalInput").ap()
    fc_b_rep = nc.dram_tensor("fc_b_rep", [P, VL], F32, kind="ExternalInput").ap()
    logits = nc.dram_tensor("logits", [RA, VL], BF16, kind="ExternalOutput").ap()

    with tile.TileContext(nc) as tc:
        with tc.tile_pool(name="dram", bufs=1, space="DRAM") as dram_pool:
            h_dram = [dram_pool.tile([P, B], BF16, name=f"hd_{t}") for t in range(T)]
            ag_outs = [dram_pool.tile([NCORES, P, B], BF16,
                                      addr_space="Shared", name=f"ag_{t}")
                       for t in range(T)]
            with tc.tile_pool(name="persist", bufs=1) as persist, \
                 tc.tile_pool(name="fc_out", bufs=3) as fc_out:
                ident_f = persist.tile([P, P], F32)
                make_identity(nc, ident_f[:])
                ident_b = persist.tile([P, P], BF16)
                nc.vector.tensor_copy(ident_b[:], ident_f[:])

                gx_sb = persist.tile([P, 4, RA], BF16)       # [gatecol, m, t*B+b]
                whh_sb = persist.tile([P, KT, GL], BF16)
                fcw_sb = persist.tile([P, KT, VL], BF16)
                fcb_sb = persist.tile([P, VL], F32)
                bias_sb = persist.tile([P, 4], F32)
                hT = [persist.tile([P, KT, B], BF16, name=f"hT_{i}")
                      for i in range(2)]
                c_sb = persist.tile([P, B], F32)

                nc.sync.dma_start(bias_sb[:], bias_t)
                for kk in range(0, KT, 4):
                    nc.gpsimd.dma_start(
                        whh_sb[:, kk:kk + 4, :],
                        whh_T.rearrange("(k p) m -> p k m", p=P)[:, kk:kk + 4, :])

                def fc_step(tt, hsrc, psum_pool, tagsfx=""):
                    for m2 in range(2):
                        row0 = tt * B + m2 * P
                        for n0, nsz in FC_CH:
                            ps = psum_pool.tile(
                                [P, 512], F32, name=f"fps{tagsfx}_{tt}_{m2}_{n0}",
                                tag=f"fps{tagsfx}")
                            for k in range(KT):
                                nc.tensor.matmul(
                                    ps[:, 0:nsz],
                                    hsrc[:, k, m2 * P:(m2 + 1) * P],
                                    fcw_sb[:, k, n0:n0 + nsz],
                                    start=(k == 0), stop=(k == KT - 1))
                            fo = fc_out.tile([P, 512], BF16,
                                             name=f"fo_{tt}_{m2}_{n0}", tag="fo")
                            nc.vector.tensor_add(
                                fo[:, 0:nsz], ps[:, 0:nsz],
                                fcb_sb[:, n0:n0 + nsz])
                            nc.scalar.dma_start(
                                logits[row0:row0 + P, n0:n0 + nsz], fo[:, 0:nsz])

                # ---- Phase A: gx_T = W_ih_own @ x^T  [512, 2560] ----
                with tc.tile_pool(name="phA", bufs=1) as phA, \
                     tc.tile_pool(name="gx_psum", bufs=4, space="PSUM") as gx_psum:
                    wih_sb = phA.tile([P, KT, GL], BF16)
                    for kk in range(0, KT, 2):
                        nc.sync.dma_start(
                            wih_sb[:, kk:kk + 2, :],
                            wih_T.rearrange("(k p) m -> p k m", p=P)[:, kk:kk + 2, :])
                    x_sb = phA.tile([P, KT, RA], BF16)
                    for n in range(NX):
                        for k in range(KT):
                            eng = nc.sync if k % 2 == 0 else nc.gpsimd
                            eng.dma_start(
                                x_sb[:, k, n * 512:(n + 1) * 512],
                                x_T[k * P:(k + 1) * P, n * 512:(n + 1) * 512])
                    # fc weights land behind x on both rings; FC(0) needs
                    # them only after the first AllGather (~95us in).
                    nc.gpsimd.dma_start(fcb_sb[:], fc_b_rep)
                    for kk in range(0, KT, 2):
                        eng = nc.sync if kk < KT // 2 else nc.gpsimd
                        eng.dma_start(
                            fcw_sb[:, kk:kk + 2, :],
                            fc_wT.rearrange("(k p) v -> p k v", p=P)[:, kk:kk + 2, :])
                    for n in range(NX):
                        for m in range(4):
                            ps = gx_psum.tile([P, 512], F32,
                                              name=f"gxps_{n}_{m}", tag="gxps")
                            for k in range(KT):
                                nc.tensor.matmul(
                                    ps[:], wih_sb[:, k, m * P:(m + 1) * P],
                                    x_sb[:, k, n * 512:(n + 1) * 512],
                                    start=(k == 0), stop=(k == KT - 1))
                            nc.vector.tensor_copy(
                                gx_sb[:, m, n * 512:(n + 1) * 512], ps[:])

                # ---- Phase B: LSTM steps + interleaved FC ----
                with tc.tile_pool(name="step_pool", bufs=2) as step_pool, \
                     tc.tile_pool(name="gps_psum", bufs=2, space="PSUM") as gps_psum, \
                     tc.tile_pool(name="fc_psum", bufs=3, space="PSUM") as fc_psum:
                    for t in range(T):
                        acts = step_pool.tile([P, 4, B], F32,
                                              name=f"acts_{t}", tag="acts")
                        tmp = step_pool.tile([P, B], F32, name=f"tmp_{t}", tag="tmp")
                        th = step_pool.tile([P, B], F32, name=f"th_{t}", tag="th")
                        h_own = step_pool.tile([P, B], BF16, name=f"h_{t}", tag="h")

                        if t == 0:
                            for m in range(4):
                                nc.scalar.activation(
                                    acts[:, m, :], gx_sb[:, m, 0:B],
                                    Act.Tanh if m == 2 else Act.Sigmoid,
                                    bias=bias_sb[:, m:m + 1])
                        else:
                            hprev = hT[(t - 1) % 2]
                            psA = gps_psum.tile([P, 512], F32,
                                                name=f"gpsA_{t}", tag="gpsA")
                            psB = gps_psum.tile([P, 512], F32,
                                                name=f"gpsB_{t}", tag="gpsB")
                            for m in range(4):
                                dst = (psA if m < 2 else psB)[
                                    :, (m % 2) * B:(m % 2) * B + B]
                                nc.vector.tensor_copy(
                                    dst, gx_sb[:, m, t * B:(t + 1) * B])
                                for k in range(KT):
                                    nc.tensor.matmul(
                                        dst, whh_sb[:, k, m * P:(m + 1) * P],
                                        hprev[:, k, :],
                                        start=False, stop=(k == KT - 1),
                                        skip_group_check=True)
                                nc.scalar.activation(
                                    acts[:, m, :], dst,
                                    Act.Tanh if m == 2 else Act.Sigmoid,
                                    bias=bias_sb[:, m:m + 1])

                        nc.vector.tensor_mul(tmp[:], acts[:, 0, :], acts[:, 2, :])
                        if t == 0:
                            nc.vector.tensor_copy(c_sb[:], tmp[:])
                        else:
                            nc.vector.tensor_mul(c_sb[:], acts[:, 1, :], c_sb[:])
                            nc.vector.tensor_add(c_sb[:], c_sb[:], tmp[:])
                        nc.scalar.activation(th[:], c_sb[:], Act.Tanh)
                        nc.vector.tensor_mul(h_own[:], acts[:, 3, :], th[:])

                        nc.scalar.dma_start(h_dram[t][:], h_own[:])
                        nc.gpsimd.collective_compute(
                            "AllGather", mybir.AluOpType.bypass,
                            replica_groups=[list(range(NCORES))],
                            ins=[h_dram[t].opt()], outs=[ag_outs[t].opt()])
                        hbuf = hT[t % 2]
                        for a in range(NCORES):
                            nc.gpsimd.dma_start(hbuf[:, a, :], ag_outs[t][a])

                        if t >= 1:
                            fc_step(t - 1, hT[(t - 1) % 2], fc_psum)
                    fc_step(T - 1, hT[(T - 1) % 2], fc_psum)
    nc.compile()
    return nc


def _build_sharded(nc, n_cores=NCORES):
    install_neuronx_cc_hook()
    partition_name = nc.partition_id_tensor.name if nc.partition_id_tensor else None
    in_names, out_names, out_avals, zero_shapes = [], [], [], []
    for alloc in nc.m.functions[0].allocations:
        if not isinstance(alloc, mybir.MemoryLocationSet):
            continue
        name = alloc.memorylocations[0].name
        if alloc.kind == "ExternalInput":
            if name != partition_name:
                in_names.append(name)
        elif alloc.kind == "ExternalOutput":
            out_names.append(name)
            shape = tuple(alloc.tensor_shape)
            dtype = mybir.dt.np(alloc.dtype)
            out_avals.append(jax.core.ShapedArray(shape, dtype))
            zero_shapes.append((shape, dtype))
    n_params = len(in_names)
    n_outs = len(out_avals)
    all_in_names = list(in_names) + list(out_names)
    if partition_name is not None:
        all_in_names.append(partition_name)
    donate = tuple(range(n_params, n_params + n_outs))

    def _body(*args):
        operands = list(args)
        if partition_name is not None:
            operands.append(partition_id_tensor())
        outs = _bass_exec_p.bind(
            *operands,
            out_avals=tuple(out_avals),
            in_names=tuple(all_in_names),
            out_names=tuple(out_names),
            lowering_input_output_aliases=(),
            sim_require_finite=True,
            sim_require_nnan=True,
            nc=nc,
        )
        return tuple(outs)

    devices = jax.devices("axon")[:n_cores]
    mesh = Mesh(np.asarray(devices), ("core",))
    in_specs = (PartitionSpec("core"),) * (n_params + n_outs)
    out_specs = (PartitionSpec("core"),) * len(out_names)
    sharded = jax.jit(
        shard_map(_body, mesh=mesh, in_specs=in_specs, out_specs=out_specs,
                  check_rep=False),
        donate_argnums=donate, keep_unused=True)

    def run(in_maps):
        concat_in = [
            np.concatenate([np.asarray(m[name]) for m in in_maps], axis=0)
            for name in in_names
        ]
        concat_zeros = [np.zeros((n_cores * s[0], *s[1:]), d) for s, d in zero_shapes]
        out_arrs = sharded(*concat_in, *concat_zeros)
        jax.block_until_ready(out_arrs)
        return [
            {name: np.asarray(out_arrs[i]).reshape(n_cores, *out_avals[i].shape)[c]
             for i, name in enumerate(out_names)}
            for c in range(n_cores)
        ]

    return run


def _prep_inputs(features, captions, emb_table, W_ih, W_hh, b_ih, b_hh, fc_W, fc_b):
    features = np.asarray(features, dtype=np.float32)
    captions = np.asarray(captions)
    emb_table = np.asarray(emb_table, dtype=np.float32)
    W_ih = np.asarray(W_ih, dtype=np.float32)
    W_hh = np.asarray(W_hh, dtype=np.float32)
    bias = (np.asarray(b_ih, dtype=np.float32) + np.asarray(b_hh, dtype=np.float32))
    fc_W = np.asarray(fc_W, dtype=np.float32)
    fc_b = np.asarray(fc_b, dtype=np.float32)

    embedded = emb_table[captions.astype(np.int64)]          # [B, T, EMB]
    lstm_in = np.concatenate([features, embedded], axis=-1)  # [B, T, DIN]
    x_T = np.ascontiguousarray(
        lstm_in.transpose(2, 1, 0).reshape(DIN, RA).astype(ml_dtypes.bfloat16))

    in_maps = []
    for c in range(NCORES):
        rows = np.concatenate(
            [g * HID + c * P + np.arange(P) for g in range(4)])   # [512]
        wih_T = np.ascontiguousarray(W_ih[rows].T.astype(ml_dtypes.bfloat16))
        whh_T = np.ascontiguousarray(W_hh[rows].T.astype(ml_dtypes.bfloat16))
        bias_t = np.ascontiguousarray(bias[rows].reshape(4, P).T)
        fc_wT = np.ascontiguousarray(
            fc_W[c * VL:(c + 1) * VL].T.astype(ml_dtypes.bfloat16))
        fcb_rep = np.ascontiguousarray(
            np.broadcast_to(fc_b[c * VL:(c + 1) * VL], (P, VL)))
        in_maps.append({
            "x_T": x_T, "wih_T": wih_T, "whh_T": whh_T, "bias_t": bias_t,
            "fc_wT": fc_wT, "fc_b_rep": fcb_rep,
        })
    return in_maps


def _unshard(results):
    out = np.empty((B, T, VOCAB), dtype=np.float32)
    for c in range(NCORES):
        out[:, :, c * VL:(c + 1) * VL] = (
            results[c]["logits"].astype(np.float32).reshape(T, B, VL).transpose(1, 0, 2))
    return out


def kernel(features, captions, emb_table, W_ih, W_hh, b_ih, b_hh, fc_W, fc_b):
    if "nc" not in _CACHE:
        _CACHE["nc"] = _build_nc()
    if "run" not in _CACHE:
        _CACHE["run"] = _build_sharded(_CACHE["nc"])
    in_maps = _prep_inputs(features, captions, emb_table, W_ih, W_hh, b_ih, b_hh,
                           fc_W, fc_b)
    results = _CACHE["run"](in_maps)
    return _unshard(results)


def kernel_traced(features, captions, emb_table, W_ih, W_hh, b_ih, b_hh, fc_W, fc_b):
    """Same computation via run_bass_kernel_spmd(trace=True); returns
    (output, BassKernelResults) so the caller can read exec_time_ns."""
    from concourse.bass_utils import run_bass_kernel_spmd
    if "nc" not in _CACHE:
        _CACHE["nc"] = _build_nc()
    in_maps = _prep_inputs(features, captions, emb_table, W_ih, W_hh, b_ih, b_hh,
                           fc_W, fc_b)
    res = run_bass_kernel_spmd(_CACHE["nc"], in_maps, list(range(NCORES)), trace=True)
    return _unshard(res.results), res



# revision 17
# speedup vs baseline: 1.0165x; 1.0165x over previous
"""Trainium2 Bass kernel for nn_Decoder (embed -> LSTM -> vocab projection).

v3 layout (8 NeuronCores, single SPMD NEFF):
  - Host: embedding gather + concat -> lstm_in [B,T,DIN]; x_T [DIN, T*B]
    (col = t*B + b) replicated to all cores; weights pre-transposed.
  - LSTM is GATE-sharded (tensor parallel): core c owns hidden slice
    c*128:(c+1)*128, i.e. 4 gate-column chunks of 128 (i,f,g,o). Per step
    the core computes gates_T[512, 256] = W_hh_own @ h (full batch, M=128
    stationary chunks, N=256 moving) -> full PE width, 33 matmuls/step.
    gx contribution folded via identity-matmul; bias via ScalarE
    activation bias (per-partition). h_T_own [128,256] needs no
    transpose (already hidden-major).
  - Per-step AllGather of h_T_own (64KB) -> h_T full [1024,256]; the FC
    for step t-1 (hs[256,1024] @ fc_W_shard^T -> [256,3750]) runs on the
    PE while the AllGather for step t is in flight.
  - gx GEMM [512, 2560] = W_ih_own @ x^T done up front (160 matmuls,
    n-major so t=0's columns finish first).
  - Output logits [T*B, VL] fp32; host re-orders (t,b) -> (b,t) and
    concats vocab shards. No row permutation games.
"""
import ml_dtypes
import numpy as np
import jax
from jax.sharding import Mesh, PartitionSpec
from jax.experimental.shard_map import shard_map

import concourse.bass as bass
import concourse.mybir as mybir
import concourse.tile as tile
from concourse import bacc
from concourse.bass2jax import _bass_exec_p, install_neuronx_cc_hook, partition_id_tensor
from concourse.masks import make_identity

P = 128
NCORES = 8
B, T, FEAT, EMB, HID, VOCAB = 256, 10, 512, 512, 1024, 30000
DIN = FEAT + EMB          # 1024
KT = DIN // P             # 8 contraction tiles (DIN and HID both 1024)
GL = 4 * P                # 512 gate columns owned per core
VL = VOCAB // NCORES      # 3750 vocab per core
RA = T * B                # 2560 output rows (t-major)
NX = RA // 512            # 5 column chunks of x / gx
F32 = mybir.dt.float32
BF16 = mybir.dt.bfloat16
Act = mybir.ActivationFunctionType

# FC vocab chunks: 7 x 512 + 166
FC_CH = [(i * 512, min(512, VL - i * 512)) for i in range((VL + 511) // 512)]

_CACHE = {}


def _build_nc():
    nc = bacc.Bacc("TRN2", target_bir_lowering=False, debug=False, num_devices=NCORES)
    x_T = nc.dram_tensor("x_T", [DIN, RA], BF16, kind="ExternalInput").ap()
    wih_T = nc.dram_tensor("wih_T", [DIN, GL], BF16, kind="ExternalInput").ap()
    whh_T = nc.dram_tensor("whh_T", [HID, GL], BF16, kind="ExternalInput").ap()
    bias_t = nc.dram_tensor("bias_t", [P, 4], F32, kind="ExternalInput").ap()
    fc_wT = nc.dram_tensor("fc_wT", [HID, VL], BF16, kind="ExternalInput").ap()
    fc_b_rep = nc.dram_tensor("fc_b_rep", [P, VL], F32, kind="ExternalInput").ap()
    logits = nc.dram_tensor("logits", [RA, VL], BF16, kind="ExternalOutput").ap()

    with tile.TileContext(nc) as tc:
        with tc.tile_pool(name="dram", bufs=1, space="DRAM") as dram_pool:
            h_dram = [dram_pool.tile([P, B], BF16, name=f"hd_{t}") for t in range(T)]
            ag_outs = [dram_pool.tile([NCORES, P, B], BF16,
                                      addr_space="Shared", name=f"ag_{t}")
                       for t in range(T)]
            with tc.tile_pool(name="persist", bufs=1) as persist, \
                 tc.tile_pool(name="fc_out", bufs=3) as fc_out:
                ident_f = persist.tile([P, P], F32)
                make_identity(nc, ident_f[:])
                ident_b = persist.tile([P, P], BF16)
                nc.vector.tensor_copy(ident_b[:], ident_f[:])

                gx_sb = persist.tile([P, 4, RA], BF16)       # [gatecol, m, t*B+b]
                whh_sb = persist.tile([P, KT, GL], BF16)
                fcw_sb = persist.tile([P, KT, VL], BF16)
                fcb_sb = persist.tile([P, VL], F32)
                bias_sb = persist.tile([P, 4], F32)
                hT = [persist.tile([P, KT, B], BF16, name=f"hT_{i}")
                      for i in range(2)]
                c_sb = persist.tile([P, B], F32)

                nc.sync.dma_start(bias_sb[:], bias_t)
                for kk in range(0, KT, 4):
                    nc.gpsimd.dma_start(
                        whh_sb[:, kk:kk + 4, :],
                        whh_T.rearrange("(k p) m -> p k m", p=P)[:, kk:kk + 4, :])

                def fc_step(tt, hsrc, psum_pool, tagsfx=""):
                    ci = 0
                    for m2 in range(2):
                        row0 = tt * B + m2 * P
                        for n0, nsz in FC_CH:
                            ps = psum_pool.tile(
                                [P, 512], F32, name=f"fps{tagsfx}_{tt}_{m2}_{n0}",
                                tag=f"fps{tagsfx}")
                            for k in range(KT):
                                nc.tensor.matmul(
                                    ps[:, 0:nsz],
                                    hsrc[:, k, m2 * P:(m2 + 1) * P],
                                    fcw_sb[:, k, n0:n0 + nsz],
                                    start=(k == 0), stop=(k == KT - 1))
                            fo = fc_out.tile([P, 512], BF16,
                                             name=f"fo_{tt}_{m2}_{n0}", tag="fo")
                            nc.vector.tensor_add(
                                fo[:, 0:nsz], ps[:, 0:nsz],
                                fcb_sb[:, n0:n0 + nsz])
                            # Alternate output rings: sync is idle once the
                            # phase-A loads drain (~86us), logits start
                            # ~112us. Halves the scalar-ring backlog and
                            # the post-final-matmul DMA tail.
                            eng = nc.scalar if ci % 2 == 0 else nc.sync
                            eng.dma_start(
                                logits[row0:row0 + P, n0:n0 + nsz], fo[:, 0:nsz])
                            ci += 1

                # ---- Phase A: gx_T = W_ih_own @ x^T  [512, 2560] ----
                with tc.tile_pool(name="phA", bufs=1) as phA, \
                     tc.tile_pool(name="gx_psum", bufs=4, space="PSUM") as gx_psum:
                    wih_sb = phA.tile([P, KT, GL], BF16)
                    for kk in range(0, KT, 2):
                        nc.sync.dma_start(
                            wih_sb[:, kk:kk + 2, :],
                            wih_T.rearrange("(k p) m -> p k m", p=P)[:, kk:kk + 2, :])
                    x_sb = phA.tile([P, KT, RA], BF16)
                    for n in range(NX):
                        for k in range(KT):
                            eng = nc.sync if k % 2 == 0 else nc.gpsimd
                            eng.dma_start(
                                x_sb[:, k, n * 512:(n + 1) * 512],
                                x_T[k * P:(k + 1) * P, n * 512:(n + 1) * 512])
                    # fc weights land behind x on both rings; FC(0) needs
                    # them only after the first AllGather (~95us in).
                    nc.gpsimd.dma_start(fcb_sb[:], fc_b_rep)
                    for kk in range(0, KT, 2):
                        eng = nc.sync if kk < KT // 2 else nc.gpsimd
                        eng.dma_start(
                            fcw_sb[:, kk:kk + 2, :],
                            fc_wT.rearrange("(k p) v -> p k v", p=P)[:, kk:kk + 2, :])
                    for n in range(NX):
                        for m in range(4):
                            ps = gx_psum.tile([P, 512], F32,
                                              name=f"gxps_{n}_{m}", tag="gxps")
                            for k in range(KT):
                                nc.tensor.matmul(
                                    ps[:], wih_sb[:, k, m * P:(m + 1) * P],
                                    x_sb[:, k, n * 512:(n + 1) * 512],
                                    start=(k == 0), stop=(k == KT - 1))
                            nc.vector.tensor_copy(
                                gx_sb[:, m, n * 512:(n + 1) * 512], ps[:])

                # ---- Phase B: LSTM steps + interleaved FC ----
                with tc.tile_pool(name="step_pool", bufs=2) as step_pool, \
                     tc.tile_pool(name="gps_psum", bufs=2, space="PSUM") as gps_psum, \
                     tc.tile_pool(name="fc_psum", bufs=3, space="PSUM") as fc_psum:
                    for t in range(T):
                        acts = step_pool.tile([P, 4, B], F32,
                                              name=f"acts_{t}", tag="acts")
                        tmp = step_pool.tile([P, B], F32, name=f"tmp_{t}", tag="tmp")
                        th = step_pool.tile([P, B], F32, name=f"th_{t}", tag="th")
                        h_own = step_pool.tile([P, B], BF16, name=f"h_{t}", tag="h")

                        if t == 0:
                            for m in range(4):
                                nc.scalar.activation(
                                    acts[:, m, :], gx_sb[:, m, 0:B],
                                    Act.Tanh if m == 2 else Act.Sigmoid,
                                    bias=bias_sb[:, m:m + 1])
                        else:
                            hprev = hT[(t - 1) % 2]
                            psA = gps_psum.tile([P, 512], F32,
                                                name=f"gpsA_{t}", tag="gpsA")
                            psB = gps_psum.tile([P, 512], F32,
                                                name=f"gpsB_{t}", tag="gpsB")
                            for m in range(4):
                                dst = (psA if m < 2 else psB)[
                                    :, (m % 2) * B:(m % 2) * B + B]
                                nc.vector.tensor_copy(
                                    dst, gx_sb[:, m, t * B:(t + 1) * B])
                                for k in range(KT):
                                    nc.tensor.matmul(
                                        dst, whh_sb[:, k, m * P:(m + 1) * P],
                                        hprev[:, k, :],
                                        start=False, stop=(k == KT - 1),
                                        skip_group_check=True)
                                nc.scalar.activation(
                                    acts[:, m, :], dst,
                                    Act.Tanh if m == 2 else Act.Sigmoid,
                                    bias=bias_sb[:, m:m + 1])

                        nc.vector.tensor_mul(tmp[:], acts[:, 0, :], acts[:, 2, :])
                        if t == 0:
                            nc.vector.tensor_copy(c_sb[:], tmp[:])
                        else:
                            nc.vector.tensor_mul(c_sb[:], acts[:, 1, :], c_sb[:])
                            nc.vector.tensor_add(c_sb[:], c_sb[:], tmp[:])
                        nc.scalar.activation(th[:], c_sb[:], Act.Tanh)
                        nc.vector.tensor_mul(h_own[:], acts[:, 3, :], th[:])

                        nc.scalar.dma_start(h_dram[t][:], h_own[:])
                        nc.gpsimd.collective_compute(
                            "AllGather", mybir.AluOpType.bypass,
                            replica_groups=[list(range(NCORES))],
                            ins=[h_dram[t].opt()], outs=[ag_outs[t].opt()])
                        hbuf = hT[t % 2]
                        for a in range(NCORES):
                            nc.gpsimd.dma_start(hbuf[:, a, :], ag_outs[t][a])

                        if t >= 1:
                            fc_step(t - 1, hT[(t - 1) % 2], fc_psum)
                    fc_step(T - 1, hT[(T - 1) % 2], fc_psum)
    nc.compile()
    return nc


def _build_sharded(nc, n_cores=NCORES):
    install_neuronx_cc_hook()
    partition_name = nc.partition_id_tensor.name if nc.partition_id_tensor else None
    in_names, out_names, out_avals, zero_shapes = [], [], [], []
    for alloc in nc.m.functions[0].allocations:
        if not isinstance(alloc, mybir.MemoryLocationSet):
            continue
        name = alloc.memorylocations[0].name
        if alloc.kind == "ExternalInput":
            if name != partition_name:
                in_names.append(name)
        elif alloc.kind == "ExternalOutput":
            out_names.append(name)
            shape = tuple(alloc.tensor_shape)
            dtype = mybir.dt.np(alloc.dtype)
            out_avals.append(jax.core.ShapedArray(shape, dtype))
            zero_shapes.append((shape, dtype))
    n_params = len(in_names)
    n_outs = len(out_avals)
    all_in_names = list(in_names) + list(out_names)
    if partition_name is not None:
        all_in_names.append(partition_name)
    donate = tuple(range(n_params, n_params + n_outs))

    def _body(*args):
        operands = list(args)
        if partition_name is not None:
            operands.append(partition_id_tensor())
        outs = _bass_exec_p.bind(
            *operands,
            out_avals=tuple(out_avals),
            in_names=tuple(all_in_names),
            out_names=tuple(out_names),
            lowering_input_output_aliases=(),
            sim_require_finite=True,
            sim_require_nnan=True,
            nc=nc,
        )
        return tuple(outs)

    devices = jax.devices("axon")[:n_cores]
    mesh = Mesh(np.asarray(devices), ("core",))
    in_specs = (PartitionSpec("core"),) * (n_params + n_outs)
    out_specs = (PartitionSpec("core"),) * len(out_names)
    sharded = jax.jit(
        shard_map(_body, mesh=mesh, in_specs=in_specs, out_specs=out_specs,
                  check_rep=False),
        donate_argnums=donate, keep_unused=True)

    def run(in_maps):
        concat_in = [
            np.concatenate([np.asarray(m[name]) for m in in_maps], axis=0)
            for name in in_names
        ]
        concat_zeros = [np.zeros((n_cores * s[0], *s[1:]), d) for s, d in zero_shapes]
        out_arrs = sharded(*concat_in, *concat_zeros)
        jax.block_until_ready(out_arrs)
        return [
            {name: np.asarray(out_arrs[i]).reshape(n_cores, *out_avals[i].shape)[c]
             for i, name in enumerate(out_names)}
            for c in range(n_cores)
        ]

    return run


def _prep_inputs(features, captions, emb_table, W_ih, W_hh, b_ih, b_hh, fc_W, fc_b):
    features = np.asarray(features, dtype=np.float32)
    captions = np.asarray(captions)
    emb_table = np.asarray(emb_table, dtype=np.float32)
    W_ih = np.asarray(W_ih, dtype=np.float32)
    W_hh = np.asarray(W_hh, dtype=np.float32)
    bias = (np.asarray(b_ih, dtype=np.float32) + np.asarray(b_hh, dtype=np.float32))
    fc_W = np.asarray(fc_W, dtype=np.float32)
    fc_b = np.asarray(fc_b, dtype=np.float32)

    embedded = emb_table[captions.astype(np.int64)]          # [B, T, EMB]
    lstm_in = np.concatenate([features, embedded], axis=-1)  # [B, T, DIN]
    x_T = np.ascontiguousarray(
        lstm_in.transpose(2, 1, 0).reshape(DIN, RA).astype(ml_dtypes.bfloat16))

    in_maps = []
    for c in range(NCORES):
        rows = np.concatenate(
            [g * HID + c * P + np.arange(P) for g in range(4)])   # [512]
        wih_T = np.ascontiguousarray(W_ih[rows].T.astype(ml_dtypes.bfloat16))
        whh_T = np.ascontiguousarray(W_hh[rows].T.astype(ml_dtypes.bfloat16))
        bias_t = np.ascontiguousarray(bias[rows].reshape(4, P).T)
        fc_wT = np.ascontiguousarray(
            fc_W[c * VL:(c + 1) * VL].T.astype(ml_dtypes.bfloat16))
        fcb_rep = np.ascontiguousarray(
            np.broadcast_to(fc_b[c * VL:(c + 1) * VL], (P, VL)))
        in_maps.append({
            "x_T": x_T, "wih_T": wih_T, "whh_T": whh_T, "bias_t": bias_t,
            "fc_wT": fc_wT, "fc_b_rep": fcb_rep,
        })
    return in_maps


def _unshard(results):
    out = np.empty((B, T, VOCAB), dtype=np.float32)
    for c in range(NCORES):
        out[:, :, c * VL:(c + 1) * VL] = (
            results[c]["logits"].astype(np.float32).reshape(T, B, VL).transpose(1, 0, 2))
    return out


def kernel(features, captions, emb_table, W_ih, W_hh, b_ih, b_hh, fc_W, fc_b):
    if "nc" not in _CACHE:
        _CACHE["nc"] = _build_nc()
    if "run" not in _CACHE:
        _CACHE["run"] = _build_sharded(_CACHE["nc"])
    in_maps = _prep_inputs(features, captions, emb_table, W_ih, W_hh, b_ih, b_hh,
                           fc_W, fc_b)
    results = _CACHE["run"](in_maps)
    return _unshard(results)


def kernel_traced(features, captions, emb_table, W_ih, W_hh, b_ih, b_hh, fc_W, fc_b):
    """Same computation via run_bass_kernel_spmd(trace=True); returns
    (output, BassKernelResults) so the caller can read exec_time_ns."""
    from concourse.bass_utils import run_bass_kernel_spmd
    if "nc" not in _CACHE:
        _CACHE["nc"] = _build_nc()
    in_maps = _prep_inputs(features, captions, emb_table, W_ih, W_hh, b_ih, b_hh,
                           fc_W, fc_b)
    res = run_bass_kernel_spmd(_CACHE["nc"], in_maps, list(range(NCORES)), trace=True)
    return _unshard(res.results), res

